# revision 18
# baseline (speedup 1.0000x reference)
"""Conv2d(32->32, 3x3, stride 1, pad 1) on X[32,32,224,224] fp32, data-parallel
over 8 NeuronCores (4 images per core).

Per-core algorithm ("image-per-row-group")
------------------------------------------
PE matmuls contract over input channels only (K=32), in float32r (single-pass
PE fp32 mode: 1 column/cycle vs 4 for exact fp32; ~1.6e-4 component rel err).

The four images owned by a core are assigned to the four 32-partition row
groups of the PE array (tile_position=(32*g, 0)).  Each image's X lives
zero-padded at partitions [32g, 32g+32) as [c, j, w] (j = padded row).  Four
K=32 matmuls - one per image - run CONCURRENTLY in the 128x128 array, each
accumulating into its own PSUM bank, so the array is fully used without any
cross-position PSUM sharing (which the walrus stack rejects at runtime).

One PSUM accumulation group = one image's output-row pair-of-pairs:
M = 64 = (ho in 0..1) x (k in 0..31), N = 448 = (u in 0..1 output-row pairs)
x (w in 0..223).  12 accumulating matmuls per group: q in 0..3 row taps
(input row j = 4P + 2u + q serves ho with r = q - ho) x s in 0..2 column
shifts (taken via the rhs free-dim offset into the padded row).

Eviction fuses the bias (ScalarE activation / VectorE tensor_scalar_add,
split per image) and places images 2,3 at SBUF partitions 64..127 (+64
shifted copies) so stores engage all 16 DMA engines.  X is processed in
four 56-output-row slices per image (SBUF fit + load/compute overlap).
"""

import sys

import numpy as np

try:
    import concourse.bass as bass  # noqa: F401
except ImportError:  # pragma: no cover
    sys.path.insert(0, "/opt/trn_rl_repo")

import concourse.mybir as mybir
import concourse.tile as tile
from concourse import bacc
from concourse.bass_utils import run_bass_kernel_spmd

NCORES = 8
NB = 4  # images per core == PE row groups
C = 32
K = 32
H = 224
W = 224
WP = 226  # padded width
RS = 56  # output rows per slice
NSLICE = H // RS
PAIRS = RS // 4  # pair-of-pairs (4 output rows) per slice
CHUNK = 2  # pairs in flight per PSUM wave (x4 images = 8 banks)
F32 = mybir.dt.float32
F32R = mybir.dt.float32r
AF = mybir.ActivationFunctionType


def conv_body(tc, X, Wt, Bias, Y):
    nc = tc.nc
    nrows = RS + 2  # padded input rows per slice
    with (
        tc.tile_pool(name="const", bufs=1) as cpool,
        tc.tile_pool(name="xpool", bufs=2) as xpool,
        tc.tile_pool(name="ypool", bufs=3) as ypool,
        tc.tile_pool(name="ppool", bufs=1, space="PSUM") as ppool,
    ):
        wt_sb = cpool.tile([128, 4, 3, 64], F32R)
        nc.sync.dma_start(out=wt_sb[:], in_=Wt)
        b_sb = cpool.tile([128, 1], F32)
        nc.sync.dma_start(out=b_sb[:], in_=Bias)
        # h = 4*P + 2*u + ho  ->  [P, ho, k, u, w] view per image for stores
        Yv = [
            Y[g].rearrange("k (P u ho) w -> P ho k u w", u=2, ho=2) for g in range(NB)
        ]

        for t in range(NSLICE):
            jA = 4 * PAIRS * t  # first padded row of the slice
            xq = xpool.tile([128, nrows, WP], F32R)
            # X arrives host-padded to [NB, C, 226, 226]: no border handling.
            for g in range(NB):
                nc.sync.dma_start(
                    out=xq[32 * g : 32 * (g + 1), :, :],
                    in_=X[g, :, jA : jA + nrows, :],
                )
            # view with j split as (jh, jl): j = 2*jh + jl;  row j=4P+2u+q sits at
            # jl=q%2, jh=2P+q//2+u  ->  u is a clean step-1 dim
            xv = xq.rearrange("p (jh jl) w -> p jl jh w", jl=2)

            for ch in range(PAIRS // CHUNK):
                pts = {}
                for p in range(CHUNK):
                    for g in range(NB):
                        pts[(g, p)] = ppool.tile(
                            [64, 2, 224], F32, name=f"pt{g}{p}", tag=f"pt{g}{p}"
                        )
                for s in range(3):
                    for q in range(4):
                        for p in range(CHUNK):
                            Ploc = ch * CHUNK + p
                            jh0 = 2 * Ploc + q // 2
                            for g in range(NB):
                                nc.tensor.matmul(
                                    pts[(g, p)][:, :, :],
                                    wt_sb[32 * g : 32 * (g + 1), q, s, :],
                                    xv[
                                        32 * g : 32 * (g + 1),
                                        q % 2,
                                        jh0 : jh0 + 2,
                                        s : s + 224,
                                    ],
                                    start=(s == 0 and q == 0),
                                    stop=(s == 2 and q == 3),
                                    tile_position=(32 * g, 0),
                                    skip_group_check=True,
                                )
                ysb = ypool.tile([128, 2, CHUNK, 2, 224], F32)
                for p in range(CHUNK):
                    for g in range(NB):
                        dst = ysb[64 * (g // 2) : 64 * (g // 2) + 64, g % 2, p, :, :]
                        src = pts[(g, p)][:, :, :]
                        if g % 2 == 0:
                            nc.scalar.activation(
                                dst, src, AF.Identity, bias=b_sb[0:64, :]
                            )
                        else:
                            nc.vector.tensor_scalar_add(dst, src, b_sb[0:64, :])
                for p in range(CHUNK):
                    Ploc = ch * CHUNK + p
                    Pglob = PAIRS * t + Ploc  # h = 4*Pglob + 2*u + ho
                    for g in range(NB):
                        for ho in range(2):
                            nc.sync.dma_start(
                                out=Yv[g][Pglob, ho],
                                in_=ysb[
                                    64 * (g // 2) + 32 * ho : 64 * (g // 2) + 32 * ho + 32,
                                    g % 2,
                                    p,
                                    :,
                                    :,
                                ],
                            )


def build_nc(nb=NB):
    assert nb == NB
    nc = bacc.Bacc("TRN2", target_bir_lowering=False, debug=False)
    X = nc.dram_tensor("X", [NB, C, H + 2, WP], F32R, kind="ExternalInput").ap()
    Wt = nc.dram_tensor("Wt", [128, 4, 3, 64], F32R, kind="ExternalInput").ap()
    Bias = nc.dram_tensor("bias", [128, 1], F32, kind="ExternalInput").ap()
    Y = nc.dram_tensor("Y", [NB, K, H, W], F32, kind="ExternalOutput").ap()
    with tile.TileContext(nc) as tc:
        conv_body(tc, X, Wt, Bias, Y)
    nc.compile()
    return nc


def prep_weights(Wf, b):
    """Wt[32*g+c, q, s, 32*ho+k] = W[k, c, q-ho, s] (0 outside 0<=r<3); same all g."""
    Wf = np.asarray(Wf, np.float32)
    Wt = np.zeros((128, 4, 3, 64), np.float32)
    for q in range(4):
        for ho in range(2):
            r = q - ho
            if 0 <= r <= 2:
                for g in range(4):
                    Wt[32 * g : 32 * g + 32, q, :, 32 * ho : 32 * ho + 32] = (
                        Wf[:, :, r, :].transpose(1, 2, 0)
                    )
    bias = np.tile(np.asarray(b, np.float32), 4).reshape(128, 1)
    return Wt, bias


_NC = None


def _get_nc():
    global _NC
    if _NC is None:
        _NC = build_nc(NB)
    return _NC


def pad_input(X):
    X = np.ascontiguousarray(X, np.float32)
    Xp = np.zeros((X.shape[0], C, H + 2, WP), np.float32)
    Xp[:, :, 1 : H + 1, 1 : W + 1] = X
    return Xp


def kernel(X, W, b, _trace=False):
    Xp = pad_input(X)
    Wt, bias = prep_weights(W, b)
    nc = _get_nc()
    in_maps = [
        {"X": Xp[NB * c : NB * (c + 1)], "Wt": Wt, "bias": bias} for c in range(NCORES)
    ]
    res = run_bass_kernel_spmd(nc, in_maps, list(range(NCORES)), trace=_trace)
    out = np.concatenate([res.results[c]["Y"] for c in range(NCORES)], axis=0)
    if _trace:
        return out, res
    return out
